# revision 2
# baseline (speedup 1.0000x reference)
"""Causal masked single-head attention [B=4, N=4096, D=768] on 8 trn2 cores.

Sharding: 2 cores per batch element; core parity c owns query blocks 2j+c
(j=0..7) of 256 rows. One SPMD instruction stream; per-core differences live
in input data (query gather order + additive causal masks).

fp8e4m3 + DoubleRow perf mode everywhere: QK^T and P@V contract 256 deep per
instruction; with FP8_PROJ the Q/K/V projections do too (weights pre-scaled
by 16 on host for fp8 range; compensated via the exp scale and a 16.0
ones-column that folds the softmax denominator into the P@V accumulation).

Host pre-packs x/xq/weights as [128, plane, cols] so each object streams in
one large DMA. Schedule: per q-block j, phase A computes scores->exp->P
(P cached in SBUF for all key-pairs of j); phase B replays P against V in
PSUM accumulators (temporal split over query halves so only 2 PSUM banks are
live). B(j-1) and projection blocks for kb=j+1 are interleaved into A(j)'s
instruction stream to fill the PE while exp chains drain.
"""

import math
import sys

sys.path.insert(0, "/opt/trn_rl_repo")

import numpy as np
import ml_dtypes

import concourse.bass as bass
import concourse.bacc as bacc
import concourse.mybir as mybir
import concourse.tile as tile
from concourse.bass_utils import run_bass_kernel_spmd

F32 = mybir.dt.float32
BF16 = mybir.dt.bfloat16
FP8 = mybir.dt.float8e4
DR = mybir.MatmulPerfMode.DoubleRow
NEG = -1.0e6
WSCALE = 16.0

FP8_PROJ = True


class Cfg:
    def __init__(self, D=768, N=4096, QB=256):
        assert D % 256 == 0 and N % 512 == 0 and QB == 256
        self.D = D
        self.N = N
        self.QB = QB
        self.QC = N // 2          # queries per core
        self.NKB = N // 512       # x stream blocks (6)
        self.NKT = N // 128       # key tiles (32)
        self.NPAIR = N // 256     # key pair-tiles (16)
        self.NQB = self.QC // QB  # local q blocks (8)
        self.OH = 384             # d_out split for PV psum
        if FP8_PROJ:
            self.scale = 1.0 / (WSCALE * WSCALE * math.sqrt(D))
        else:
            self.scale = 1.0 / math.sqrt(D)


def build_kernel(cfg: Cfg, repeat: int = 1) -> bass.Bass:
    import contextlib
    D, N, QB, OH = cfg.D, cfg.N, cfg.QB, cfg.OH
    xdt = FP8 if FP8_PROJ else BF16
    nc = bacc.Bacc("TRN2")

    # all host-side packed as [128, plane, cols]
    xT = nc.dram_tensor("xT", [128, 6, N], xdt, kind="ExternalInput")
    xqT = nc.dram_tensor("xqT", [128, 6, cfg.QC], xdt, kind="ExternalInput")
    wqT = nc.dram_tensor("wqT", [128, 6, D], xdt, kind="ExternalInput")
    wkT = nc.dram_tensor("wkT", [128, 6, D], xdt, kind="ExternalInput")
    wvT = nc.dram_tensor("wvT", [128, 6, D], xdt, kind="ExternalInput")
    mask4 = nc.dram_tensor("mask4", [128, 4, QB], BF16, kind="ExternalInput")
    # precise (bf16) path inputs for the shortest softmax rows (q block j=0)
    xbf = nc.dram_tensor("xbf", [128, 6, 512], BF16, kind="ExternalInput")
    xqbf = nc.dram_tensor("xqbf", [128, 6, QB], BF16, kind="ExternalInput")
    wqB = nc.dram_tensor("wqB", [128, 6, D], BF16, kind="ExternalInput")
    wkB = nc.dram_tensor("wkB", [128, 6, D], BF16, kind="ExternalInput")
    wvB = nc.dram_tensor("wvB", [128, 6, D], BF16, kind="ExternalInput")
    out = nc.dram_tensor("out", [cfg.QC, D], BF16, kind="ExternalOutput")

    with tile.TileContext(nc) as tc:
        with (
            tc.tile_pool(name="persist", bufs=1) as persist,
            tc.tile_pool(name="xstream", bufs=2) as xstream,
            tc.tile_pool(name="ptp", bufs=2) as ptp,
            tc.tile_pool(name="work", bufs=3) as work,
            tc.tile_pool(name="sc", bufs=3, space="PSUM") as scp,
            tc.tile_pool(name="ops", bufs=2, space="PSUM") as opsp,
        ):
            # persistent SBUF: weights [128, 6, D] (plane = d_in slice)
            wq_w = persist.tile([128, 6, D], xdt, tag="wq", name="wq")
            wk_w = persist.tile([128, 6, D], xdt, tag="wk", name="wk")
            wv_w = persist.tile([128, 6, D], xdt, tag="wv", name="wv")
            KTP = [persist.tile([128, 2, N], FP8, tag=f"KTP{t}", name=f"KTP{t}")
                   for t in range(3)]
            QTP = [persist.tile([128, 2, cfg.QC], FP8, tag=f"QTP{t}", name=f"QTP{t}")
                   for t in range(3)]
            VP = [persist.tile([128, 2, D + 1], FP8, tag=f"VP{t}", name=f"VP{t}")
                  for t in range(cfg.NPAIR)]
            msk = persist.tile([128, 4, QB], BF16, tag="msk", name="msk")
            # precise-path SBUF
            wqb_w = persist.tile([128, 6, D], BF16, tag="wqb", name="wqb")
            wkb_w = persist.tile([128, 6, D], BF16, tag="wkb", name="wkb")
            wvb_w = persist.tile([128, 6, D], BF16, tag="wvb", name="wvb")
            xbf_w = persist.tile([128, 6, 512], BF16, tag="xbfw", name="xbfw")
            xqb_w = persist.tile([128, 6, QB], BF16, tag="xqbw", name="xqbw")
            KB = persist.tile([128, 6, 512], BF16, tag="KB", name="KB")
            QB_ = persist.tile([128, 6, QB], BF16, tag="QB_", name="QB_")
            VB = persist.tile([128, 4, D + 1], BF16, tag="VB", name="VB")

            rep_ctx = tc.For_i(0, repeat, 1) if repeat > 1 else contextlib.nullcontext()
            with rep_ctx:
                # ---- emission helpers ----
                def load_x(src, cols, c0, tag):
                    tl = xstream.tile([128, 6, 512], xdt, tag=tag, name=tag)
                    nc.sync.dma_start(tl[:, :, :], src[:, :, c0:c0 + 512])
                    return tl

                def cp_dve(dst, src):
                    nc.vector.tensor_copy(dst, src)

                def cp_act(dst, src):
                    nc.scalar.copy(dst, src)

                def kq_group(xb, w_w, dst, tp, dstc, cp):
                    """K^T/Q^T out planes (2tp, 2tp+1), 512 cols -> one copy."""
                    ps = scp.tile([128, 2, 512], F32, tag="sc", name="sc")
                    for i in range(2):
                        ot = 2 * tp + i
                        if FP8_PROJ:
                            for t in range(3):
                                nc.tensor.matmul(
                                    ps[:, i:i + 1, :],
                                    w_w[:, 2 * t:2 * t + 2, 128 * ot:128 * (ot + 1)],
                                    xb[:, 2 * t:2 * t + 2, :],
                                    start=(t == 0), stop=(t == 2), perf_mode=DR)
                        else:
                            for t in range(6):
                                nc.tensor.matmul(
                                    ps[:, i:i + 1, :],
                                    w_w[:, t:t + 1, 128 * ot:128 * (ot + 1)],
                                    xb[:, t:t + 1, :],
                                    start=(t == 0), stop=(t == 5))
                    cp(dst[tp][:, :, dstc:dstc + 512], ps[:, :, :])

                def v_group(xb, kt, cp):
                    """V[kt] full row block: 2 oh halves -> one copy."""
                    ps = scp.tile([128, 2, 512], F32, tag="sc", name="sc")
                    kl = kt % 4
                    for oh in range(2):
                        if FP8_PROJ:
                            for t in range(3):
                                nc.tensor.matmul(
                                    ps[:, oh:oh + 1, 0:OH],
                                    xb[:, 2 * t:2 * t + 2, 128 * kl:128 * (kl + 1)],
                                    wv_w[:, 2 * t:2 * t + 2, OH * oh:OH * (oh + 1)],
                                    start=(t == 0), stop=(t == 2), perf_mode=DR)
                        else:
                            for t in range(6):
                                nc.tensor.matmul(
                                    ps[:, oh:oh + 1, 0:OH],
                                    xb[:, t:t + 1, 128 * kl:128 * (kl + 1)],
                                    wv_w[:, t:t + 1, OH * oh:OH * (oh + 1)],
                                    start=(t == 0), stop=(t == 5))
                    cp(VP[kt // 2][:, (kt % 2):(kt % 2) + 1, 0:D],
                       ps[:, 0:2, 0:OH])

                def proj_thunks(kb):
                    xb = load_x(xT, N, 512 * kb, "xb")
                    th = []
                    for tp in range(3):
                        th.append(lambda tp=tp: kq_group(
                            xb, wk_w, KTP, tp, 512 * kb, cp_dve))
                    for kl in range(4):
                        th.append(lambda kl=kl: v_group(xb, 4 * kb + kl, cp_act))
                    return th

                def qproj_thunks(qb):
                    xq = load_x(xqT, cfg.QC, 512 * qb, "xq")
                    return [
                        lambda tp=tp: kq_group(xq, wq_w, QTP, tp, 512 * qb, cp_dve)
                        for tp in range(3)
                    ]

                def emit_scores(j, p, pt):
                    st = scp.tile([128, 2, 512], F32, tag="sc", name="sc")
                    for i in range(2):
                        kt = 2 * p + i
                        for t in range(3):
                            nc.tensor.matmul(
                                st[:, i:i + 1, 0:QB],
                                KTP[t][:, :, 128 * kt:128 * (kt + 1)],
                                QTP[t][:, :, QB * j:QB * (j + 1)],
                                start=(t == 0), stop=(t == 2), perf_mode=DR)
                    mp = p - 2 * j
                    if mp >= 0:
                        nc.vector.tensor_add(st[:, 0:2, 0:QB], st[:, 0:2, 0:QB],
                                             msk[:, 2 * mp:2 * mp + 2, :])
                    nc.scalar.activation(pt[:, 0:2, :], st[:, 0:2, 0:QB],
                                         mybir.ActivationFunctionType.Exp,
                                         scale=cfg.scale)

                def precise_thunks():
                    """bf16 path for q block j=0 (rows with <=512 keys)."""
                    th = []

                    def kq_b(tp, w_w, dst, mv, ncols):
                        ps = scp.tile([128, 2, 512], F32, tag="sc", name="sc")
                        for i in range(2):
                            ot = 2 * tp + i
                            for t in range(6):
                                nc.tensor.matmul(
                                    ps[:, i:i + 1, 0:ncols],
                                    w_w[:, t:t + 1, 128 * ot:128 * (ot + 1)],
                                    mv[:, t:t + 1, :],
                                    start=(t == 0), stop=(t == 5))
                        cp_dve(dst[:, 2 * tp:2 * tp + 2, :], ps[:, 0:2, 0:ncols])

                    def v_b(kt):
                        ps = scp.tile([128, 2, 512], F32, tag="sc", name="sc")
                        for oh in range(2):
                            for t in range(6):
                                nc.tensor.matmul(
                                    ps[:, oh:oh + 1, 0:OH],
                                    xbf_w[:, t:t + 1, 128 * kt:128 * (kt + 1)],
                                    wvb_w[:, t:t + 1, OH * oh:OH * (oh + 1)],
                                    start=(t == 0), stop=(t == 5))
                        cp_act(VB[:, kt:kt + 1, 0:D], ps[:, 0:2, 0:OH])

                    for tp in range(3):
                        th.append(lambda tp=tp: kq_b(tp, wkb_w, KB, xbf_w, 512))
                    for kt in range(4):
                        th.append(lambda kt=kt: v_b(kt))
                    for tp in range(3):
                        th.append(lambda tp=tp: kq_b(tp, wqb_w, QB_, xqb_w, QB))

                    pbs = [ptp.tile([128, 2, QB], BF16, tag=f"pb{p}", name=f"pb{p}")
                           for p in range(2)]

                    def sc_b(p):
                        st = scp.tile([128, 2, 512], F32, tag="sc", name="sc")
                        for i in range(2):
                            kt = 2 * p + i
                            for t in range(6):
                                nc.tensor.matmul(
                                    st[:, i:i + 1, 0:QB],
                                    KB[:, t:t + 1, 128 * kt:128 * (kt + 1)],
                                    QB_[:, t:t + 1, :],
                                    start=(t == 0), stop=(t == 5))
                        nc.vector.tensor_add(st[:, 0:2, 0:QB], st[:, 0:2, 0:QB],
                                             msk[:, 2 * p:2 * p + 2, :])
                        nc.scalar.activation(pbs[p][:, 0:2, :], st[:, 0:2, 0:QB],
                                             mybir.ActivationFunctionType.Exp,
                                             scale=1.0 / math.sqrt(D))
                    th.append(lambda: sc_b(0))
                    th.append(lambda: sc_b(1))

                    for qh in range(2):
                        ops = [None, None]

                        def alloc(ops=ops):
                            ops[0] = opsp.tile([128, 512], F32, tag="ops", name="ops")
                            ops[1] = opsp.tile([128, 512], F32, tag="ops", name="ops")
                        th.append(alloc)
                        for oh in range(2):
                            w = OH + (1 if oh == 1 else 0)
                            for kt in range(4):
                                def pv(qh=qh, oh=oh, kt=kt, w=w, ops=ops):
                                    nc.tensor.matmul(
                                        ops[oh][:, 0:w],
                                        pbs[kt // 2][:, (kt % 2):(kt % 2) + 1,
                                                     128 * qh:128 * (qh + 1)],
                                        VB[:, kt:kt + 1, OH * oh:OH * oh + w],
                                        start=(kt == 0), stop=(kt == 3))
                                th.append(pv)

                        def fin(qh=qh, ops=ops):
                            rec = work.tile([128, 1], F32, tag="rec", name="rec")
                            nc.vector.reciprocal(rec[:], ops[1][:, OH:OH + 1])
                            osb = work.tile([128, D], BF16, tag="osb", name="osb")
                            nc.vector.tensor_scalar_mul(
                                osb[:, 0:OH], ops[0][:, 0:OH], rec[:])
                            nc.vector.tensor_scalar_mul(
                                osb[:, OH:D], ops[1][:, 0:OH], rec[:])
                            nc.sync.dma_start(
                                out[128 * qh:128 * (qh + 1), :], osb[:])
                        th.append(fin)
                    return th

                def b_thunks(j, pts):
                    npair = 2 * j + 2
                    th = []
                    for qh in range(2):
                        ops = [None, None]

                        def alloc(ops=ops):
                            ops[0] = opsp.tile([128, 512], F32, tag="ops", name="ops")
                            ops[1] = opsp.tile([128, 512], F32, tag="ops", name="ops")
                        th.append(alloc)
                        for oh in range(2):
                            w = OH + (1 if oh == 1 else 0)
                            for p in range(npair):
                                def pv(qh=qh, oh=oh, p=p, w=w, ops=ops, npair=npair):
                                    nc.tensor.matmul(
                                        ops[oh][:, 0:w],
                                        pts[p][:, :, 128 * qh:128 * (qh + 1)],
                                        VP[p][:, :, OH * oh:OH * oh + w],
                                        start=(p == 0), stop=(p == npair - 1),
                                        perf_mode=DR)
                                th.append(pv)

                        def fin(j=j, qh=qh, ops=ops):
                            rec = work.tile([128, 1], F32, tag="rec", name="rec")
                            nc.vector.reciprocal(rec[:], ops[1][:, OH:OH + 1])
                            osb = work.tile([128, D], BF16, tag="osb", name="osb")
                            nc.vector.tensor_scalar_mul(
                                osb[:, 0:OH], ops[0][:, 0:OH], rec[:])
                            nc.vector.tensor_scalar_mul(
                                osb[:, OH:D], ops[1][:, 0:OH], rec[:])
                            nc.sync.dma_start(
                                out[QB * j + 128 * qh:QB * j + 128 * (qh + 1), :],
                                osb[:])
                        th.append(fin)
                    return th

                # ---- prologue ----
                nc.sync.dma_start(wk_w[:, :, :], wkT[:, :, :])
                pre = proj_thunks(0)   # also starts xb(0) DMA
                nc.sync.dma_start(wv_w[:, :, :], wvT[:, :, :])
                nc.sync.dma_start(wq_w[:, :, :], wqT[:, :, :])
                nc.sync.dma_start(msk[:, :, :], mask4[:, :, :])
                nc.scalar.dma_start(wkb_w[:, :, :], wkB[:, :, :])
                nc.scalar.dma_start(wvb_w[:, :, :], wvB[:, :, :])
                nc.scalar.dma_start(wqb_w[:, :, :], wqB[:, :, :])
                nc.scalar.dma_start(xbf_w[:, :, :], xbf[:, :, :])
                nc.scalar.dma_start(xqb_w[:, :, :], xqbf[:, :, :])
                ones = WSCALE if FP8_PROJ else 1.0
                for t in range(cfg.NPAIR):
                    for i in range(2):
                        nc.gpsimd.memset(VP[t][:, i:i + 1, D:D + 1], ones)
                for kt in range(4):
                    nc.gpsimd.memset(VB[:, kt:kt + 1, D:D + 1], 1.0)
                for th in pre:
                    th()
                for th in qproj_thunks(0):
                    th()
                for th in proj_thunks(1):
                    th()

                # ---- main schedule (j=0 runs as the precise bf16 path) ----
                prev_pts = None
                for j in range(1, cfg.NQB):
                    if j == 1:
                        fill = qproj_thunks(1)
                    else:
                        fill = b_thunks(j - 1, prev_pts)
                        if j == 2:
                            fill += precise_thunks()
                        if j in (3, 5):
                            fill += qproj_thunks(j // 2 + 1)
                    if j + 1 < cfg.NKB:
                        fill += proj_thunks(j + 1)

                    npair = 2 * j + 2
                    pts = [ptp.tile([128, 2, QB], FP8, tag=f"pt{p}", name=f"pt{p}")
                           for p in range(npair)]
                    per = -(-len(fill) // npair)  # ceil
                    fi = 0
                    for p in range(npair):
                        emit_scores(j, p, pts[p])
                        for _ in range(per):
                            if fi < len(fill):
                                fill[fi]()
                                fi += 1
                    while fi < len(fill):
                        fill[fi]()
                        fi += 1
                    prev_pts = pts

                for th in b_thunks(cfg.NQB - 1, prev_pts):
                    th()
    nc.compile()
    return nc


# ---------------------------------------------------------------------------
# Host-side sharding / gather
# ---------------------------------------------------------------------------


def pack_rows(a: np.ndarray, dt) -> np.ndarray:
    """[768, C] -> [128, 6, C]: plane t holds rows 128t..128t+127."""
    return np.ascontiguousarray(
        a.reshape(6, 128, a.shape[1]).transpose(1, 0, 2)).astype(dt)


def make_masks(QB: int, parity: int) -> np.ndarray:
    kk = np.arange(128)[:, None]
    qq = np.arange(QB)[None, :]
    tri0 = np.where(kk <= qq, 0.0, NEG).astype(np.float32)
    tri1 = np.where(kk + 128 <= qq, 0.0, NEG).astype(np.float32)
    zero = np.zeros((128, QB), np.float32)
    full = np.full((128, QB), NEG, np.float32)
    blocks = [tri0, tri1, full, full] if parity == 0 else [zero, zero, tri0, tri1]
    return np.stack(blocks, axis=1).astype(ml_dtypes.bfloat16)


def core_inputs(cfg: Cfg, x_b: np.ndarray, WqT, WkT, WvT, WB, parity: int) -> dict:
    dt = ml_dtypes.float8_e4m3 if FP8_PROJ else ml_dtypes.bfloat16
    bfdt = ml_dtypes.bfloat16
    QB = cfg.QB
    xT = pack_rows(x_b.T, dt)
    cols = []
    for j in range(cfg.NQB):
        gb = 2 * j + parity
        cols.append(x_b[QB * gb:QB * (gb + 1), :].T)
    xqT = pack_rows(np.concatenate(cols, axis=1), dt)
    return {
        "xT": xT,
        "xqT": xqT,
        "wqT": WqT,
        "wkT": WkT,
        "wvT": WvT,
        "mask4": make_masks(QB, parity),
        "xbf": pack_rows(x_b[0:512, :].T, bfdt),
        "xqbf": pack_rows(x_b[QB * parity:QB * (parity + 1), :].T, bfdt),
        "wqB": WB[0],
        "wkB": WB[1],
        "wvB": WB[2],
    }


def scatter_output(cfg: Cfg, out_core: np.ndarray, parity: int,
                   dst: np.ndarray) -> None:
    QB = cfg.QB
    for j in range(cfg.NQB):
        gb = 2 * j + parity
        dst[QB * gb:QB * (gb + 1), :] = out_core[QB * j:QB * (j + 1), :].astype(
            np.float32)


def build_in_maps(cfg: Cfg, input_batch, Wq, Wk, Wv):
    dt = ml_dtypes.float8_e4m3 if FP8_PROJ else ml_dtypes.bfloat16
    bfdt = ml_dtypes.bfloat16
    s = WSCALE if FP8_PROJ else 1.0
    x = np.asarray(input_batch, dtype=np.float32)
    WqT = pack_rows(s * np.asarray(Wq, np.float32).T, dt)
    WkT = pack_rows(s * np.asarray(Wk, np.float32).T, dt)
    WvT = pack_rows(s * np.asarray(Wv, np.float32).T, dt)
    WB = [pack_rows(np.asarray(W, np.float32).T, bfdt) for W in (Wq, Wk, Wv)]
    return [core_inputs(cfg, x[c // 2], WqT, WkT, WvT, WB, c % 2) for c in range(8)]


_CACHE: dict = {}


def _get_nc(cfg: Cfg) -> bass.Bass:
    if "nc" not in _CACHE:
        _CACHE["nc"] = build_kernel(cfg)
    return _CACHE["nc"]


def kernel(input_batch, Wq, Wk, Wv):
    cfg = Cfg()
    nc = _get_nc(cfg)
    in_maps = build_in_maps(cfg, input_batch, Wq, Wk, Wv)
    res = run_bass_kernel_spmd(nc, in_maps, core_ids=list(range(8)))
    B = np.asarray(input_batch).shape[0]
    out = np.empty((B, cfg.N, cfg.D), np.float32)
    for c in range(2 * B):
        scatter_output(cfg, res.results[c]["out"], c % 2, out[c // 2])
    return out
